# revision 13
# baseline (speedup 1.0000x reference)
"""MHA kernel for trn2: 8-core SPMD, core c = (batch c//2, head-group c%2 of 8 heads).

Per-core pipeline (all shapes hardcoded for B=4, S=2048, HIDDEN=1024, H=16, DK=DV=64):
  Phase 1: Q^T/K^T per head-pair [128, S] and ones-augmented V [sk, 8, 65] via PE,
           biases folded in as K=1 matmuls.
  Phase 2: per sq-block j (512) per head: scores^T = K Q^T (K=64 matmuls),
           exp on ACT (scale=1/8), mask multiply on DVE (bf16 mask),
           PV accumulation with augmented V -> row 64 = softmax denominator.
           Normalize via DVE reciprocal + K=1 matmul partition broadcast.
  Phase 3 (interleaved per j): out-projection with pair-packed lhsT (K=128),
           partial output [S, 1024] per core; host sums the 2 groups + bo.
"""

import numpy as np
import ml_dtypes

import concourse.bacc as bacc
import concourse.mybir as mybir
import concourse.tile as tile
from concourse.bass_utils import run_bass_kernel_spmd

B, S, HID, H = 4, 2048, 1024, 16
DK = DV = 64
G = 2              # head groups per batch (8 heads each)
HPC, PAIRS = 8, 4  # heads / head-pairs per core
SQB = 512          # sq block
NJ = S // SQB      # 4
NT = S // 128      # 16 sk tiles
KTN = HID // 128   # 8 hidden k-tiles

F32 = mybir.dt.float32
BF16 = mybir.dt.bfloat16
AF = mybir.ActivationFunctionType

_NC = None


def _build_nc():
    nc = bacc.Bacc("TRN2")
    xq_d = nc.declare_dram_parameter("xqT", [HID, S], F32, isOutput=False)
    xk_d = nc.declare_dram_parameter("xkT", [HID, S], F32, isOutput=False)
    xv_d = nc.declare_dram_parameter("xvT", [HID, S], F32, isOutput=False)
    mk_d = nc.declare_dram_parameter("maskJ", [NJ, S, SQB], BF16, isOutput=False)
    wq_d = nc.declare_dram_parameter("wq", [HID, 512], F32, isOutput=False)
    wk_d = nc.declare_dram_parameter("wk", [HID, 512], F32, isOutput=False)
    wv_d = nc.declare_dram_parameter("wv", [HID, 512], F32, isOutput=False)
    bq_d = nc.declare_dram_parameter("bq", [1, 512], F32, isOutput=False)
    bk_d = nc.declare_dram_parameter("bk", [1, 512], F32, isOutput=False)
    bv_d = nc.declare_dram_parameter("bv", [1, 512], F32, isOutput=False)
    wo_d = nc.declare_dram_parameter("wo", [PAIRS, 128, HID], F32, isOutput=False)
    out_d = nc.declare_dram_parameter("out", [S, HID], F32, isOutput=True)

    with tile.TileContext(nc) as tc:
        with tc.tile_pool(name="persist", bufs=1) as PP:
            qT = PP.tile([128, PAIRS, S], F32, name="qT")
            kT = PP.tile([128, PAIRS, S], F32, name="kT")
            vA = PP.tile([128, NT, HPC, DV + 1], F32, name="vA")
            ones = PP.tile([1, SQB], F32, name="ones")
            bq_sb = PP.tile([1, 512], F32, name="bq_sb")
            bk_sb = PP.tile([1, 512], F32, name="bk_sb")
            bv_sb = PP.tile([1, 512], F32, name="bv_sb")
            nc.vector.memset(ones[:], 1.0)
            nc.vector.memset(vA[:, :, :, DV:DV + 1], 1.0)
            nc.sync.dma_start(bq_sb[:], bq_d[:])
            nc.sync.dma_start(bk_sb[:], bk_d[:])
            nc.sync.dma_start(bv_sb[:], bv_d[:])

            # ---------------- Phase 1: projections ----------------
            with tc.tile_pool(name="xpool", bufs=2) as XP, \
                 tc.tile_pool(name="wpool", bufs=2) as WP, \
                 tc.tile_pool(name="ph1ps", bufs=2, space="PSUM") as PR:
                # Q and K: out[pair-dk 128, sq] += w.T @ x, bias via K=1 matmul
                for xd, wd, brow, dstT in ((xq_d, wq_d, bq_sb, qT),
                                           (xk_d, wk_d, bk_sb, kT)):
                    w_sb = WP.tile([128, KTN, 512], F32, name="w_sb")
                    for k in range(KTN):
                        nc.sync.dma_start(w_sb[:, k, :], wd[k * 128:(k + 1) * 128, :])
                    for n in range(NJ):
                        x_sb = XP.tile([128, KTN, SQB], F32, name="x_sb")
                        for k in range(KTN):
                            nc.sync.dma_start(
                                x_sb[:, k, :],
                                xd[k * 128:(k + 1) * 128, n * SQB:(n + 1) * SQB])
                        for hp in range(PAIRS):
                            ps = PR.tile([128, SQB], F32, name="ps_qk")
                            for k in range(KTN):
                                nc.tensor.matmul(
                                    ps[:], w_sb[:, k, hp * 128:(hp + 1) * 128],
                                    x_sb[:, k, :], start=(k == 0), stop=False)
                            nc.tensor.matmul(
                                ps[:], brow[0:1, hp * 128:(hp + 1) * 128],
                                ones[0:1, :], start=False, stop=True)
                            nc.scalar.activation(
                                dstT[:, hp, n * SQB:(n + 1) * SQB], ps[:], AF.Copy)

                # V: out[sk 128, head, dv] += x.T @ wv, bias via K=1 matmul
                wv_sb = WP.tile([128, KTN, 512], F32, name="w_sb")
                for k in range(KTN):
                    nc.sync.dma_start(wv_sb[:, k, :], wv_d[k * 128:(k + 1) * 128, :])
                for c in range(4):
                    x_sb = XP.tile([128, KTN, SQB], F32, name="x_sb")
                    for k in range(KTN):
                        nc.sync.dma_start(
                            x_sb[:, k, :],
                            xv_d[k * 128:(k + 1) * 128, c * SQB:(c + 1) * SQB])
                    for stl in range(4):
                        st = c * 4 + stl
                        ps = PR.tile([128, HPC, DV], F32, name="ps_v")
                        for k in range(KTN):
                            nc.tensor.matmul(
                                ps[:], x_sb[:, k, stl * 128:(stl + 1) * 128],
                                wv_sb[:, k, :], start=(k == 0), stop=False)
                        nc.tensor.matmul(ps[:], ones[0:1, 0:128], bv_sb[0:1, :],
                                         start=False, stop=True)
                        nc.vector.tensor_copy(vA[:, st, :, 0:DV], ps[:])

            # ---------------- Phase 2+3: attention + out-projection ----------------
            with tc.tile_pool(name="oTp", bufs=1) as OP, \
                 tc.tile_pool(name="wop", bufs=1) as WOP, \
                 tc.tile_pool(name="mskp", bufs=2) as MP, \
                 tc.tile_pool(name="ptp", bufs=2) as PTP, \
                 tc.tile_pool(name="rcp", bufs=2) as RCP, \
                 tc.tile_pool(name="obp", bufs=2) as OBP, \
                 tc.tile_pool(name="scps", bufs=2, space="PSUM") as SCP, \
                 tc.tile_pool(name="pvps", bufs=2, space="PSUM") as PVP, \
                 tc.tile_pool(name="opps", bufs=2, space="PSUM") as OPP:
                oT = OP.tile([128, PAIRS, S], F32, name="oT")
                wo_sb = WOP.tile([128, PAIRS, HID], F32, name="wo_sb")
                for hp in range(PAIRS):
                    nc.sync.dma_start(wo_sb[:, hp, :], wo_d[hp])

                for j in range(NJ):
                    msk = MP.tile([128, NT, SQB], BF16, name="msk")
                    nc.sync.dma_start(
                        msk[:], mk_d[j].rearrange("(t p) s -> p t s", p=128))
                    for hl in range(HPC):
                        hp, r = divmod(hl, 2)
                        pb = 64 * r
                        pv = PVP.tile([DV + 1, SQB], F32, name="pv")
                        for tt in range(NT // 2):
                            sc = SCP.tile([128, 2, SQB], F32, name="sc")
                            for u in range(2):
                                t = 2 * tt + u
                                nc.tensor.matmul(
                                    sc[:, u, :],
                                    kT[pb:pb + DK, hp, t * 128:(t + 1) * 128],
                                    qT[pb:pb + DK, hp, j * SQB:(j + 1) * SQB],
                                    start=True, stop=True)
                            pt = PTP.tile([128, 2, SQB], F32, name="pt")
                            nc.scalar.activation(pt[:], sc[:], AF.Exp, scale=0.125)
                            nc.vector.tensor_mul(pt[:], pt[:],
                                                 msk[:, 2 * tt:2 * tt + 2, :])
                            for u in range(2):
                                t = 2 * tt + u
                                nc.tensor.matmul(
                                    pv[:], vA[:, t, hl, :], pt[:, u, :],
                                    start=(tt == 0 and u == 0),
                                    stop=(tt == NT // 2 - 1 and u == 1))
                        rc = RCP.tile([1, SQB], F32, name="rc")
                        nc.vector.reciprocal(rc[:], pv[DV:DV + 1, :])
                        bc = OPP.tile([128, SQB], F32, name="op")
                        nc.tensor.matmul(bc[0:DV, :], ones[0:1, 0:DV],
                                         rc[0:1, :], start=True, stop=True)
                        bc_sb = OBP.tile([128, SQB], F32, name="ob")
                        nc.scalar.activation(bc_sb[0:DV, :], bc[0:DV, :],
                                             AF.Copy)
                        nc.vector.tensor_mul(
                            oT[pb:pb + DV, hp, j * SQB:(j + 1) * SQB],
                            pv[0:DV, :], bc_sb[0:DV, :])
                    # out-projection for this j's 4 sq-tiles
                    for stl in range(4):
                        st = 4 * j + stl
                        for nn in range(2):
                            op = OPP.tile([128, SQB], F32, name="op")
                            for hp in range(PAIRS):
                                nc.tensor.matmul(
                                    op[:], oT[:, hp, st * 128:(st + 1) * 128],
                                    wo_sb[:, hp, nn * SQB:(nn + 1) * SQB],
                                    start=(hp == 0), stop=(hp == PAIRS - 1))
                            ob = OBP.tile([128, SQB], F32, name="ob")
                            nc.scalar.activation(ob[:], op[:], AF.Copy)
                            nc.sync.dma_start(
                                out_d[st * 128:(st + 1) * 128,
                                      nn * SQB:(nn + 1) * SQB], ob[:])
    nc.finalize()
    return nc


def get_nc():
    global _NC
    if _NC is None:
        _NC = _build_nc()
    return _NC


def make_in_maps(q_hidden_inputs, k_hidden_inputs, v_hidden_inputs, mask,
                 wq, bq, wk, bk, wv, bv, wo, bo):
    f32 = np.float32
    in_maps = []
    per_batch = []
    for b in range(B):
        xqT = np.ascontiguousarray(q_hidden_inputs[b].T, dtype=f32)
        xkT = np.ascontiguousarray(k_hidden_inputs[b].T, dtype=f32)
        xvT = np.ascontiguousarray(v_hidden_inputs[b].T, dtype=f32)
        maskT = mask[b].T.astype(ml_dtypes.bfloat16)          # [sk, sq]
        maskJ = np.ascontiguousarray(
            maskT.reshape(S, NJ, SQB).transpose(1, 0, 2))     # [j, sk, 512]
        per_batch.append((xqT, xkT, xvT, maskJ))
    for c in range(2 * B):
        b, g = divmod(c, 2)
        xqT, xkT, xvT, maskJ = per_batch[b]
        hs = slice(g * HPC, (g + 1) * HPC)
        in_maps.append({
            "xqT": xqT, "xkT": xkT, "xvT": xvT, "maskJ": maskJ,
            "wq": np.ascontiguousarray(
                wq[hs].transpose(1, 0, 2).reshape(HID, 512), dtype=f32),
            "wk": np.ascontiguousarray(
                wk[hs].transpose(1, 0, 2).reshape(HID, 512), dtype=f32),
            "wv": np.ascontiguousarray(
                wv[hs].transpose(1, 0, 2).reshape(HID, 512), dtype=f32),
            "bq": np.ascontiguousarray(bq[hs].reshape(1, 512), dtype=f32),
            "bk": np.ascontiguousarray(bk[hs].reshape(1, 512), dtype=f32),
            "bv": np.ascontiguousarray(bv[hs].reshape(1, 512), dtype=f32),
            "wo": np.ascontiguousarray(
                wo[g * 512:(g + 1) * 512, :].reshape(PAIRS, 128, HID), dtype=f32),
        })
    return in_maps


def assemble(results, bo):
    out = np.empty((B, S, HID), dtype=np.float32)
    for b in range(B):
        out[b] = results[2 * b]["out"] + results[2 * b + 1]["out"] \
            + bo.astype(np.float32)[None, :]
    return out


def run(inputs, trace=False, **kw):
    nc = get_nc()
    in_maps = make_in_maps(**inputs)
    bkr = run_bass_kernel_spmd(nc, in_maps, list(range(2 * B)), trace=trace, **kw)
    return assemble(bkr.results, np.asarray(inputs["bo"])), bkr


def kernel(**inputs):
    out, _ = run(inputs, trace=False)
    return out


# revision 24
# speedup vs baseline: 2.0806x; 2.0806x over previous
"""MHA kernel for trn2: 8-core SPMD, core c = (batch c//2, head-group c%2 of 8 heads).

Per-core pipeline (all shapes hardcoded for B=4, S=2048, HIDDEN=1024, H=16, DK=DV=64):
  Phase 1: Q^T/K^T per head-pair [128, S] and ones-augmented V [sk, 8, 65] via PE,
           biases folded in as K=1 matmuls.
  Phase 2: per sq-block j (512) per head: scores^T = K Q^T (K=64 matmuls),
           exp on ACT (scale=1/8), mask multiply on DVE (bf16 mask),
           PV accumulation with augmented V -> row 64 = softmax denominator.
           Normalize via DVE reciprocal + K=1 matmul partition broadcast.
  Phase 3 (interleaved per j): out-projection with pair-packed lhsT (K=128),
           partial output [S, 1024] per core; host sums the 2 groups + bo.
"""

import numpy as np
import ml_dtypes

import concourse.bacc as bacc
import concourse.mybir as mybir
import concourse.tile as tile
from concourse.bass_utils import run_bass_kernel_spmd

B, S, HID, H = 4, 2048, 1024, 16
DK = DV = 64
G = 2              # head groups per batch (8 heads each)
HPC, PAIRS = 8, 4  # heads / head-pairs per core
SQB = 512          # sq block
NJ = S // SQB      # 4
NT = S // 128      # 16 sk tiles
KTN = HID // 128   # 8 hidden k-tiles

F32 = mybir.dt.float32
F32R = mybir.dt.float32r
BF16 = mybir.dt.bfloat16
AF = mybir.ActivationFunctionType


_NC = None


def _build_nc():
    nc = bacc.Bacc("TRN2")
    xq_d = nc.declare_dram_parameter("xqT", [HID, S], F32R, isOutput=False)
    xk_d = nc.declare_dram_parameter("xkT", [HID, S], F32R, isOutput=False)
    xv_d = nc.declare_dram_parameter("xvT", [HID, S], F32R, isOutput=False)
    mk_d = nc.declare_dram_parameter("maskJ", [NJ, S, SQB], BF16, isOutput=False)
    wq_d = nc.declare_dram_parameter("wq", [HID, 512], F32R, isOutput=False)
    wk_d = nc.declare_dram_parameter("wk", [HID, 512], F32R, isOutput=False)
    wv_d = nc.declare_dram_parameter("wv", [HID, 512], F32R, isOutput=False)
    bq_d = nc.declare_dram_parameter("bq", [1, 512], F32R, isOutput=False)
    bk_d = nc.declare_dram_parameter("bk", [1, 512], F32R, isOutput=False)
    bv_d = nc.declare_dram_parameter("bv", [1, 512], F32R, isOutput=False)
    wo_d = nc.declare_dram_parameter("wo", [PAIRS, 128, HID], F32, isOutput=False)
    out_d = nc.declare_dram_parameter("out", [S, HID], F32, isOutput=True)

    with tile.TileContext(nc) as tc:
        with tc.tile_pool(name="persist", bufs=1) as PP:
            qT = PP.tile([128, PAIRS, S], F32R, name="qT")
            kT = PP.tile([128, PAIRS, S], F32R, name="kT")
            vA = PP.tile([128, NT, HPC, DV + 1], BF16, name="vA")
            ones = PP.tile([1, SQB], F32R, name="ones")
            bq_sb = PP.tile([1, 512], F32R, name="bq_sb")
            bk_sb = PP.tile([1, 512], F32R, name="bk_sb")
            bv_sb = PP.tile([1, 512], F32R, name="bv_sb")
            ones_f = PP.tile([1, SQB], F32, name="ones_f")
            nc.vector.memset(ones_f[:], 1.0)
            nc.scalar.activation(ones[:], ones_f[:], AF.Copy)
            nc.vector.memset(vA[:, :, :, DV:DV + 1], 1.0)
            nc.sync.dma_start(bq_sb[:], bq_d[:])
            nc.sync.dma_start(bk_sb[:], bk_d[:])
            nc.sync.dma_start(bv_sb[:], bv_d[:])

            # ---------------- Phase 1: projections ----------------
            with tc.tile_pool(name="xpool", bufs=2) as XP, \
                 tc.tile_pool(name="wpool", bufs=2) as WP, \
                 tc.tile_pool(name="ph1ps", bufs=2, space="PSUM") as PR:
                # Q and K: out[pair-dk 128, sq] += w.T @ x, bias via K=1 matmul
                for xd, wd, brow, dstT in ((xq_d, wq_d, bq_sb, qT),
                                           (xk_d, wk_d, bk_sb, kT)):
                    w_sb = WP.tile([128, KTN, 512], F32R, name="w_sb")
                    for k in range(KTN):
                        nc.sync.dma_start(w_sb[:, k, :], wd[k * 128:(k + 1) * 128, :])
                    for n in range(NJ):
                        x_sb = XP.tile([128, KTN, SQB], F32R, name="x_sb")
                        for k in range(KTN):
                            nc.sync.dma_start(
                                x_sb[:, k, :],
                                xd[k * 128:(k + 1) * 128, n * SQB:(n + 1) * SQB])
                        for hp in range(PAIRS):
                            ps = PR.tile([128, SQB], F32, name="ps_qk")
                            for k in range(KTN):
                                nc.tensor.matmul(
                                    ps[:], w_sb[:, k, hp * 128:(hp + 1) * 128],
                                    x_sb[:, k, :], start=(k == 0), stop=False)
                            nc.tensor.matmul(
                                ps[:], brow[0:1, hp * 128:(hp + 1) * 128],
                                ones[0:1, :], start=False, stop=True)
                            nc.scalar.activation(
                                dstT[:, hp, n * SQB:(n + 1) * SQB], ps[:], AF.Copy)

                # V: out[sk 128, head, dv] += x.T @ wv, bias via K=1 matmul
                wv_sb = WP.tile([128, KTN, 512], F32R, name="w_sb")
                for k in range(KTN):
                    nc.sync.dma_start(wv_sb[:, k, :], wv_d[k * 128:(k + 1) * 128, :])
                for c in range(4):
                    x_sb = XP.tile([128, KTN, SQB], F32R, name="x_sb")
                    for k in range(KTN):
                        nc.sync.dma_start(
                            x_sb[:, k, :],
                            xv_d[k * 128:(k + 1) * 128, c * SQB:(c + 1) * SQB])
                    for stl in range(4):
                        st = c * 4 + stl
                        ps = PR.tile([128, HPC, DV], F32, name="ps_v")
                        for k in range(KTN):
                            nc.tensor.matmul(
                                ps[:], x_sb[:, k, stl * 128:(stl + 1) * 128],
                                wv_sb[:, k, :], start=(k == 0), stop=False)
                        nc.tensor.matmul(ps[:], ones[0:1, 0:128],
                                         bv_sb[0:1, :],
                                         start=False, stop=True)
                        nc.vector.tensor_copy(vA[:, st, :, 0:DV], ps[:])

            # ---------------- Phase 2+3: attention + out-projection ----------------
            with tc.tile_pool(name="oTp", bufs=1) as OP, \
                 tc.tile_pool(name="wop", bufs=1) as WOP, \
                 tc.tile_pool(name="mskp", bufs=2) as MP, \
                 tc.tile_pool(name="ptp", bufs=2) as PTP, \
                 tc.tile_pool(name="rcp", bufs=2) as RCP, \
                 tc.tile_pool(name="obp", bufs=2) as OBP, \
                 tc.tile_pool(name="scps", bufs=2, space="PSUM") as SCP, \
                 tc.tile_pool(name="pvps", bufs=2, space="PSUM") as PVP, \
                 tc.tile_pool(name="opps", bufs=2, space="PSUM") as OPP:
                oT = OP.tile([128, PAIRS, S], BF16, name="oT")
                wo_sb = WOP.tile([128, PAIRS, HID], BF16, name="wo_sb")
                with tc.tile_pool(name="wofp", bufs=2) as WFP:
                    for hp in range(PAIRS):
                        wo_f = WFP.tile([128, HID], F32, name="wo_f")
                        nc.sync.dma_start(wo_f[:], wo_d[hp])
                        nc.scalar.activation(wo_sb[:, hp, :], wo_f[:], AF.Copy)

                for j in range(NJ):
                    msk = MP.tile([128, NT, SQB], BF16, name="msk")
                    nc.sync.dma_start(
                        msk[:], mk_d[j].rearrange("(t p) s -> p t s", p=128))
                    for hl in range(HPC):
                        hp, r = divmod(hl, 2)
                        pb = 64 * r
                        pv = PVP.tile([DV + 1, SQB], F32, name="pv")
                        for tt in range(NT // 2):
                            sc = SCP.tile([128, 2, SQB], F32, name="sc")
                            for u in range(2):
                                t = 2 * tt + u
                                nc.tensor.matmul(
                                    sc[:, u, :],
                                    kT[pb:pb + DK, hp, t * 128:(t + 1) * 128],
                                    qT[pb:pb + DK, hp, j * SQB:(j + 1) * SQB],
                                    start=True, stop=True)
                            pt = PTP.tile([128, 2, SQB], BF16, name="pt")
                            nc.scalar.activation(pt[:], sc[:], AF.Exp, scale=0.125)
                            nc.vector.tensor_mul(pt[:], pt[:],
                                                 msk[:, 2 * tt:2 * tt + 2, :])
                            for u in range(2):
                                t = 2 * tt + u
                                nc.tensor.matmul(
                                    pv[:], vA[:, t, hl, :], pt[:, u, :],
                                    start=(tt == 0 and u == 0),
                                    stop=(tt == NT // 2 - 1 and u == 1))
                        rc = RCP.tile([1, SQB], F32R, name="rc")
                        with nc.allow_low_precision(reason="f32r recip"):
                            nc.vector.reciprocal(rc[:], pv[DV:DV + 1, :])
                        bc = OPP.tile([128, SQB], F32, name="op")
                        nc.tensor.matmul(bc[0:DV, :], ones[0:1, 0:DV],
                                         rc[0:1, :], start=True, stop=True)
                        bc_sb = OBP.tile([128, SQB], F32, name="ob")
                        nc.scalar.activation(bc_sb[0:DV, :], bc[0:DV, :],
                                             AF.Copy)
                        nc.vector.tensor_mul(
                            oT[pb:pb + DV, hp, j * SQB:(j + 1) * SQB],
                            pv[0:DV, :], bc_sb[0:DV, :])
                    # out-projection for this j's 4 sq-tiles
                    for stl in range(4):
                        st = 4 * j + stl
                        for nn in range(2):
                            op = OPP.tile([128, SQB], F32, name="op")
                            for hp in range(PAIRS):
                                nc.tensor.matmul(
                                    op[:], oT[:, hp, st * 128:(st + 1) * 128],
                                    wo_sb[:, hp, nn * SQB:(nn + 1) * SQB],
                                    start=(hp == 0), stop=(hp == PAIRS - 1))
                            ob = OBP.tile([128, SQB], F32, name="ob")
                            nc.scalar.activation(ob[:], op[:], AF.Copy)
                            nc.sync.dma_start(
                                out_d[st * 128:(st + 1) * 128,
                                      nn * SQB:(nn + 1) * SQB], ob[:])
    nc.finalize()
    return nc


def get_nc():
    global _NC
    if _NC is None:
        _NC = _build_nc()
    return _NC


def make_in_maps(q_hidden_inputs, k_hidden_inputs, v_hidden_inputs, mask,
                 wq, bq, wk, bk, wv, bv, wo, bo):
    f32 = np.float32
    in_maps = []
    per_batch = []
    for b in range(B):
        xqT = np.ascontiguousarray(q_hidden_inputs[b].T, dtype=f32)
        xkT = np.ascontiguousarray(k_hidden_inputs[b].T, dtype=f32)
        xvT = np.ascontiguousarray(v_hidden_inputs[b].T, dtype=f32)
        maskT = mask[b].T.astype(ml_dtypes.bfloat16)          # [sk, sq]
        maskJ = np.ascontiguousarray(
            maskT.reshape(S, NJ, SQB).transpose(1, 0, 2))     # [j, sk, 512]
        per_batch.append((xqT, xkT, xvT, maskJ))
    for c in range(2 * B):
        b, g = divmod(c, 2)
        xqT, xkT, xvT, maskJ = per_batch[b]
        hs = slice(g * HPC, (g + 1) * HPC)
        in_maps.append({
            "xqT": xqT, "xkT": xkT, "xvT": xvT, "maskJ": maskJ,
            "wq": np.ascontiguousarray(
                wq[hs].transpose(1, 0, 2).reshape(HID, 512), dtype=f32),
            "wk": np.ascontiguousarray(
                wk[hs].transpose(1, 0, 2).reshape(HID, 512), dtype=f32),
            "wv": np.ascontiguousarray(
                wv[hs].transpose(1, 0, 2).reshape(HID, 512), dtype=f32),
            "bq": np.ascontiguousarray(bq[hs].reshape(1, 512), dtype=f32),
            "bk": np.ascontiguousarray(bk[hs].reshape(1, 512), dtype=f32),
            "bv": np.ascontiguousarray(bv[hs].reshape(1, 512), dtype=f32),
            "wo": np.ascontiguousarray(
                wo[g * 512:(g + 1) * 512, :].reshape(PAIRS, 128, HID), dtype=f32),
        })
    return in_maps


def assemble(results, bo):
    out = np.empty((B, S, HID), dtype=np.float32)
    for b in range(B):
        out[b] = results[2 * b]["out"] + results[2 * b + 1]["out"] \
            + bo.astype(np.float32)[None, :]
    return out


def run(inputs, trace=False, **kw):
    nc = get_nc()
    in_maps = make_in_maps(**inputs)
    bkr = run_bass_kernel_spmd(nc, in_maps, list(range(2 * B)), trace=trace, **kw)
    return assemble(bkr.results, np.asarray(inputs["bo"])), bkr


def kernel(**inputs):
    out, _ = run(inputs, trace=False)
    return out


# revision 34
# speedup vs baseline: 2.6582x; 1.2776x over previous
"""MHA kernel for trn2: 8-core SPMD, core c = (batch c//2, head-group c%2 of 8 heads).

Per-core pipeline (all shapes hardcoded for B=4, S=2048, HIDDEN=1024, H=16, DK=DV=64):
  Phase 1: Q^T/K^T per head-pair [128, S] and ones-augmented V [sk, 8, 65] via PE,
           biases folded in as K=1 matmuls.
  Phase 2: per sq-block j (512) per head: scores^T = K Q^T (K=64 matmuls),
           exp on ACT (scale=1/8), mask multiply on DVE (bf16 mask),
           PV accumulation with augmented V -> row 64 = softmax denominator.
           Normalize via DVE reciprocal + K=1 matmul partition broadcast.
  Phase 3 (interleaved per j): out-projection with pair-packed lhsT (K=128),
           partial output [S, 1024] per core; host sums the 2 groups + bo.
"""

import numpy as np
import ml_dtypes

import concourse.bacc as bacc
import concourse.mybir as mybir
import concourse.tile as tile
from concourse.bass_utils import run_bass_kernel_spmd

B, S, HID, H = 4, 2048, 1024, 16
DK = DV = 64
G = 2              # head groups per batch (8 heads each)
HPC, PAIRS = 8, 4  # heads / head-pairs per core
SQB = 512          # sq block
NJ = S // SQB      # 4
NT = S // 128      # 16 sk tiles
KTN = HID // 128   # 8 hidden k-tiles

F32 = mybir.dt.float32
F32R = mybir.dt.float32r
BF16 = mybir.dt.bfloat16
AF = mybir.ActivationFunctionType


_NC = None


def _build_nc():
    nc = bacc.Bacc("TRN2")
    xq_d = nc.declare_dram_parameter("xqT", [HID, S], F32R, isOutput=False)
    xk_d = nc.declare_dram_parameter("xkT", [HID, S], F32R, isOutput=False)
    xv_d = nc.declare_dram_parameter("xvT", [HID, S], F32R, isOutput=False)
    mk_d = nc.declare_dram_parameter("maskJ", [NJ, S, SQB], BF16, isOutput=False)
    wq_d = nc.declare_dram_parameter("wq", [HID, 512], F32R, isOutput=False)
    wk_d = nc.declare_dram_parameter("wk", [HID, 512], F32R, isOutput=False)
    wv_d = nc.declare_dram_parameter("wv", [HID, 512], F32R, isOutput=False)
    bq_d = nc.declare_dram_parameter("bq", [1, 512], F32R, isOutput=False)
    bk_d = nc.declare_dram_parameter("bk", [1, 512], F32R, isOutput=False)
    bv_d = nc.declare_dram_parameter("bv", [1, 512], F32R, isOutput=False)
    wo_d = nc.declare_dram_parameter("wo", [PAIRS, 128, HID], F32, isOutput=False)
    sel_d = nc.declare_dram_parameter("sel", [HPC, HPC * DV], F32R, isOutput=False)
    out_d = nc.declare_dram_parameter("out", [S, HID], F32, isOutput=True)

    with tile.TileContext(nc) as tc:
        with tc.tile_pool(name="persist", bufs=1) as PP:
            qT = PP.tile([128, PAIRS, S], F32R, name="qT")
            kT = PP.tile([128, PAIRS, S], F32R, name="kT")
            vA = PP.tile([128, NT, HPC, DV + 1], BF16, name="vA")
            ones = PP.tile([1, SQB], F32R, name="ones")
            bq_sb = PP.tile([1, 512], F32R, name="bq_sb")
            bk_sb = PP.tile([1, 512], F32R, name="bk_sb")
            bv_sb = PP.tile([1, 512], F32R, name="bv_sb")
            ones_f = PP.tile([1, SQB], F32, name="ones_f")
            nc.vector.memset(ones_f[:], 1.0)
            nc.scalar.activation(ones[:], ones_f[:], AF.Copy)
            nc.vector.memset(vA[:, :, :, DV:DV + 1], 1.0)
            # one-hot selector rows: sel[k, hl*64:(hl+1)*64] = (k == hl)
            sel = PP.tile([HPC, HPC * DV], F32R, name="sel")
            nc.sync.dma_start(sel[:], sel_d[:])
            nc.sync.dma_start(bq_sb[:], bq_d[:])
            nc.sync.dma_start(bk_sb[:], bk_d[:])
            nc.sync.dma_start(bv_sb[:], bv_d[:])

            # ---------------- Phase 1: projections ----------------
            with tc.tile_pool(name="xpool", bufs=2) as XP, \
                 tc.tile_pool(name="wpool", bufs=2) as WP, \
                 tc.tile_pool(name="ph1ps", bufs=2, space="PSUM") as PR:
                # Q and K: out[pair-dk 128, sq] += w.T @ x, bias via K=1 matmul
                for xd, wd, brow, dstT in ((xq_d, wq_d, bq_sb, qT),
                                           (xk_d, wk_d, bk_sb, kT)):
                    w_sb = WP.tile([128, KTN, 512], F32R, name="w_sb")
                    for k in range(KTN):
                        nc.sync.dma_start(w_sb[:, k, :], wd[k * 128:(k + 1) * 128, :])
                    for n in range(NJ):
                        x_sb = XP.tile([128, KTN, SQB], F32R, name="x_sb")
                        for k in range(KTN):
                            nc.sync.dma_start(
                                x_sb[:, k, :],
                                xd[k * 128:(k + 1) * 128, n * SQB:(n + 1) * SQB])
                        for hp in range(PAIRS):
                            ps = PR.tile([128, SQB], F32, name="ps_qk")
                            for k in range(KTN):
                                nc.tensor.matmul(
                                    ps[:], w_sb[:, k, hp * 128:(hp + 1) * 128],
                                    x_sb[:, k, :], start=(k == 0), stop=False)
                            nc.tensor.matmul(
                                ps[:], brow[0:1, hp * 128:(hp + 1) * 128],
                                ones[0:1, :], start=False, stop=True)
                            nc.scalar.activation(
                                dstT[:, hp, n * SQB:(n + 1) * SQB], ps[:], AF.Copy)

                # V: out[sk 128, head, dv] += x.T @ wv, bias via K=1 matmul
                wv_sb = WP.tile([128, KTN, 512], F32R, name="w_sb")
                for k in range(KTN):
                    nc.sync.dma_start(wv_sb[:, k, :], wv_d[k * 128:(k + 1) * 128, :])
                for c in range(4):
                    x_sb = XP.tile([128, KTN, SQB], F32R, name="x_sb")
                    for k in range(KTN):
                        nc.sync.dma_start(
                            x_sb[:, k, :],
                            xv_d[k * 128:(k + 1) * 128, c * SQB:(c + 1) * SQB])
                    for stl in range(4):
                        st = c * 4 + stl
                        ps = PR.tile([128, HPC, DV], F32, name="ps_v")
                        for k in range(KTN):
                            nc.tensor.matmul(
                                ps[:], x_sb[:, k, stl * 128:(stl + 1) * 128],
                                wv_sb[:, k, :], start=(k == 0), stop=False)
                        nc.tensor.matmul(ps[:], ones[0:1, 0:128],
                                         bv_sb[0:1, :],
                                         start=False, stop=True)
                        nc.vector.tensor_copy(vA[:, st, :, 0:DV], ps[:])

            # ---------------- Phase 2+3: attention + out-projection ----------------
            with tc.tile_pool(name="wop", bufs=1) as WOP, \
                 tc.tile_pool(name="mskp", bufs=2) as MP, \
                 tc.tile_pool(name="ptp", bufs=3) as PTP, \
                 tc.tile_pool(name="oup", bufs=2) as OUP, \
                 tc.tile_pool(name="onp", bufs=2) as ONP, \
                 tc.tile_pool(name="dnp", bufs=2) as DNP, \
                 tc.tile_pool(name="dtp", bufs=2) as DTP, \
                 tc.tile_pool(name="rcp", bufs=2) as RCP, \
                 tc.tile_pool(name="obp", bufs=2) as OBP, \
                 tc.tile_pool(name="scps", bufs=2, space="PSUM") as SCP, \
                 tc.tile_pool(name="pvps", bufs=2, space="PSUM") as PVP, \
                 tc.tile_pool(name="opps", bufs=2, space="PSUM") as OPP:
                wo_sb = WOP.tile([128, PAIRS, HID], BF16, name="wo_sb")
                with tc.tile_pool(name="wofp", bufs=2) as WFP:
                    for hp in range(PAIRS):
                        wo_f = WFP.tile([128, HID], F32, name="wo_f")
                        nc.sync.dma_start(wo_f[:], wo_d[hp])
                        nc.scalar.activation(wo_sb[:, hp, :], wo_f[:], AF.Copy)

                def emit_tail(j, denj, oU, oN):
                    # batched reciprocal over all 8 heads' denominators
                    rc8 = RCP.tile([HPC, SQB], F32R, name="rc8")
                    with nc.allow_low_precision(reason="f32r recip"):
                        nc.vector.reciprocal(rc8[:], denj[:])
                    for hl in range(HPC):
                        hp, r = divmod(hl, 2)
                        pb = 64 * r
                        bc = OPP.tile([128, SQB], F32, name="op")
                        nc.tensor.matmul(bc[0:DV, :],
                                         sel[:, hl * DV:(hl + 1) * DV],
                                         rc8[:], start=True, stop=True)
                        nc.vector.tensor_mul(oN[pb:pb + DV, hp, :],
                                             oU[pb:pb + DV, hp, :], bc[0:DV, :])
                    for stl in range(4):
                        st = 4 * j + stl
                        for nn in range(2):
                            op = OPP.tile([128, SQB], F32, name="op")
                            for hp in range(PAIRS):
                                nc.tensor.matmul(
                                    op[:], oN[:, hp, stl * 128:(stl + 1) * 128],
                                    wo_sb[:, hp, nn * SQB:(nn + 1) * SQB],
                                    start=(hp == 0), stop=(hp == PAIRS - 1))
                            ob = OBP.tile([128, SQB], F32, name="ob")
                            nc.scalar.activation(ob[:], op[:], AF.Copy)
                            nc.sync.dma_start(
                                out_d[st * 128:(st + 1) * 128,
                                      nn * SQB:(nn + 1) * SQB], ob[:])

                pending = None
                for j in range(NJ):
                    msk = MP.tile([128, NT, SQB], BF16, name="msk")
                    nc.sync.dma_start(
                        msk[:], mk_d[j].rearrange("(t p) s -> p t s", p=128))
                    denj = DNP.tile([HPC, SQB], F32R, name="denj")
                    oU = OUP.tile([128, PAIRS, SQB], BF16, name="oU")
                    oN = ONP.tile([128, PAIRS, SQB], BF16, name="oN")
                    for hl in range(HPC):
                        hp, r = divmod(hl, 2)
                        pb = 64 * r
                        pv = PVP.tile([DV + 1, SQB], F32, name="pv")
                        # software pipeline: scores MMs for tt enqueue on the PE
                        # before PV MMs for tt-1, so the PE never head-of-line
                        # blocks on exp/mask of the previous tile.
                        prev_pt = None
                        for tt in range(NT // 2):
                            sc = SCP.tile([128, 2, SQB], F32, name="sc")
                            for u in range(2):
                                t = 2 * tt + u
                                nc.tensor.matmul(
                                    sc[:, u, :],
                                    kT[pb:pb + DK, hp, t * 128:(t + 1) * 128],
                                    qT[pb:pb + DK, hp, j * SQB:(j + 1) * SQB],
                                    start=True, stop=True)
                            if prev_pt is not None:
                                ppt, ptt = prev_pt
                                for u in range(2):
                                    nc.tensor.matmul(
                                        pv[:], vA[:, 2 * ptt + u, hl, :],
                                        ppt[:, u, :],
                                        start=(ptt == 0 and u == 0), stop=False)
                            pt = PTP.tile([128, 2, SQB], BF16, name="pt")
                            nc.scalar.activation(pt[:], sc[:], AF.Exp, scale=0.125)
                            nc.vector.tensor_mul(pt[:], pt[:],
                                                 msk[:, 2 * tt:2 * tt + 2, :])
                            prev_pt = (pt, tt)
                        ppt, ptt = prev_pt
                        for u in range(2):
                            nc.tensor.matmul(
                                pv[:], vA[:, 2 * ptt + u, hl, :], ppt[:, u, :],
                                start=False, stop=(u == 1))
                        dtmp = DTP.tile([1, SQB], F32R, name="dtmp")
                        with nc.allow_low_precision(reason="den f32r"):
                            nc.vector.tensor_copy(dtmp[:], pv[DV:DV + 1, :])
                        nc.sync.dma_start(denj[hl:hl + 1, :], dtmp[:])
                        nc.vector.tensor_copy(oU[pb:pb + DV, hp, :], pv[0:DV, :])
                        if hl == 0 and pending is not None:
                            emit_tail(*pending)
                            pending = None
                    pending = (j, denj, oU, oN)
                emit_tail(*pending)
    nc.finalize()
    return nc


def get_nc():
    global _NC
    if _NC is None:
        _NC = _build_nc()
    return _NC


def make_in_maps(q_hidden_inputs, k_hidden_inputs, v_hidden_inputs, mask,
                 wq, bq, wk, bk, wv, bv, wo, bo):
    f32 = np.float32
    in_maps = []
    per_batch = []
    sel = np.zeros((HPC, HPC * DV), dtype=f32)
    for hl in range(HPC):
        sel[hl, hl * DV:(hl + 1) * DV] = 1.0
    for b in range(B):
        xqT = np.ascontiguousarray(q_hidden_inputs[b].T, dtype=f32)
        xkT = np.ascontiguousarray(k_hidden_inputs[b].T, dtype=f32)
        xvT = np.ascontiguousarray(v_hidden_inputs[b].T, dtype=f32)
        maskT = mask[b].T.astype(ml_dtypes.bfloat16)          # [sk, sq]
        maskJ = np.ascontiguousarray(
            maskT.reshape(S, NJ, SQB).transpose(1, 0, 2))     # [j, sk, 512]
        per_batch.append((xqT, xkT, xvT, maskJ))
    for c in range(2 * B):
        b, g = divmod(c, 2)
        xqT, xkT, xvT, maskJ = per_batch[b]
        hs = slice(g * HPC, (g + 1) * HPC)
        in_maps.append({
            "xqT": xqT, "xkT": xkT, "xvT": xvT, "maskJ": maskJ,
            "wq": np.ascontiguousarray(
                wq[hs].transpose(1, 0, 2).reshape(HID, 512), dtype=f32),
            "wk": np.ascontiguousarray(
                wk[hs].transpose(1, 0, 2).reshape(HID, 512), dtype=f32),
            "wv": np.ascontiguousarray(
                wv[hs].transpose(1, 0, 2).reshape(HID, 512), dtype=f32),
            "bq": np.ascontiguousarray(bq[hs].reshape(1, 512), dtype=f32),
            "bk": np.ascontiguousarray(bk[hs].reshape(1, 512), dtype=f32),
            "bv": np.ascontiguousarray(bv[hs].reshape(1, 512), dtype=f32),
            "wo": np.ascontiguousarray(
                wo[g * 512:(g + 1) * 512, :].reshape(PAIRS, 128, HID), dtype=f32),
            "sel": sel,
        })
    return in_maps


def assemble(results, bo):
    out = np.empty((B, S, HID), dtype=np.float32)
    for b in range(B):
        out[b] = results[2 * b]["out"] + results[2 * b + 1]["out"] \
            + bo.astype(np.float32)[None, :]
    return out


def run(inputs, trace=False, **kw):
    nc = get_nc()
    in_maps = make_in_maps(**inputs)
    bkr = run_bass_kernel_spmd(nc, in_maps, list(range(2 * B)), trace=trace, **kw)
    return assemble(bkr.results, np.asarray(inputs["bo"])), bkr


def kernel(**inputs):
    out, _ = run(inputs, trace=False)
    return out


# revision 39
# speedup vs baseline: 3.0004x; 1.1287x over previous
"""MHA kernel for trn2: 8-core SPMD, core c = (batch c//2, head-group c%2 of 8 heads).

Per-core pipeline (all shapes hardcoded for B=4, S=2048, HIDDEN=1024, H=16, DK=DV=64):
  Phase 1: Q^T/K^T per head-pair [128, S] (bf16) and ones-augmented V [sk, 8, 65]
           via PE, biases folded in as K=1 matmuls.
  Phase 2: per sq-block j (512) per head: scores^T = K Q^T (K=64 matmuls),
           exp on ACT (scale=1/8), mask multiply on DVE (bf16 mask),
           PV accumulation with augmented V -> row 64 = softmax denominator.
  Phase 3 (software-pipelined into the next j block): batched reciprocal,
           per-head normalization via one-hot-selector matmul broadcast + DVE
           multiply, out-projection with pair-packed lhsT (K=128); the tail is
           drip-fed one step per score tile to keep the PE free of stalls.
  Host sums the 2 group partials per batch + bo.
"""

import numpy as np
import ml_dtypes

import concourse.bacc as bacc
import concourse.mybir as mybir
import concourse.tile as tile
from concourse.bass_utils import run_bass_kernel_spmd

B, S, HID, H = 4, 2048, 1024, 16
DK = DV = 64
G = 2              # head groups per batch (8 heads each)
HPC, PAIRS = 8, 4  # heads / head-pairs per core
SQB = 512          # sq block
NJ = S // SQB      # 4
NT = S // 128      # 16 sk tiles
KTN = HID // 128   # 8 hidden k-tiles

F32 = mybir.dt.float32
F32R = mybir.dt.float32r
BF16 = mybir.dt.bfloat16
AF = mybir.ActivationFunctionType


_NC = None


def _build_nc():
    nc = bacc.Bacc("TRN2")
    xq_d = nc.declare_dram_parameter("xqT", [HID, S], F32R, isOutput=False)
    xk_d = nc.declare_dram_parameter("xkT", [HID, S], F32R, isOutput=False)
    xv_d = nc.declare_dram_parameter("xvT", [HID, S], F32R, isOutput=False)
    mk_d = nc.declare_dram_parameter("maskJ", [NJ, S, SQB], BF16, isOutput=False)
    wq_d = nc.declare_dram_parameter("wq", [HID, 512], F32R, isOutput=False)
    wk_d = nc.declare_dram_parameter("wk", [HID, 512], F32R, isOutput=False)
    wv_d = nc.declare_dram_parameter("wv", [HID, 512], F32R, isOutput=False)
    bq_d = nc.declare_dram_parameter("bq", [1, 512], F32R, isOutput=False)
    bk_d = nc.declare_dram_parameter("bk", [1, 512], F32R, isOutput=False)
    bv_d = nc.declare_dram_parameter("bv", [1, 512], F32R, isOutput=False)
    wo_d = nc.declare_dram_parameter("wo", [PAIRS, 128, HID], BF16, isOutput=False)
    sel_d = nc.declare_dram_parameter("sel", [HPC, HPC * DV], F32R, isOutput=False)
    out_d = nc.declare_dram_parameter("out", [S, HID], F32, isOutput=True)

    with tile.TileContext(nc) as tc:
        with tc.tile_pool(name="persist", bufs=1) as PP, \
             tc.tile_pool(name="wop", bufs=1) as WOP, \
             tc.tile_pool(name="mskp", bufs=2) as MP, \
             tc.tile_pool(name="ptp", bufs=3) as PTP, \
             tc.tile_pool(name="oup", bufs=2) as OUP, \
             tc.tile_pool(name="onp", bufs=2) as ONP, \
             tc.tile_pool(name="dnp", bufs=2) as DNP, \
             tc.tile_pool(name="dtp", bufs=2) as DTP, \
             tc.tile_pool(name="rcp", bufs=2) as RCP, \
             tc.tile_pool(name="obp", bufs=2) as OBP:
            qT = PP.tile([128, PAIRS, S], BF16, name="qT")
            kT = PP.tile([128, PAIRS, S], BF16, name="kT")
            vA = PP.tile([128, NT, HPC, DV + 1], BF16, name="vA")
            ones = PP.tile([1, SQB], F32R, name="ones")
            bq_sb = PP.tile([1, 512], F32R, name="bq_sb")
            bk_sb = PP.tile([1, 512], F32R, name="bk_sb")
            bv_sb = PP.tile([1, 512], F32R, name="bv_sb")
            ones_f = PP.tile([1, SQB], F32, name="ones_f")
            nc.vector.memset(ones_f[:], 1.0)
            nc.scalar.activation(ones[:], ones_f[:], AF.Copy)
            nc.vector.memset(vA[:, :, :, DV:DV + 1], 1.0)
            # one-hot selector rows: sel[k, hl*64:(hl+1)*64] = (k == hl)
            sel = PP.tile([HPC, HPC * DV], F32R, name="sel")
            nc.sync.dma_start(sel[:], sel_d[:])
            nc.sync.dma_start(bq_sb[:], bq_d[:])
            nc.sync.dma_start(bk_sb[:], bk_d[:])
            nc.sync.dma_start(bv_sb[:], bv_d[:])
            wo_sb = WOP.tile([128, PAIRS, HID], BF16, name="wo_sb")
            for hp in range(PAIRS):
                nc.sync.dma_start(wo_sb[:, hp, :], wo_d[hp])
            msk0 = MP.tile([128, NT, SQB], BF16, name="msk")
            nc.sync.dma_start(
                msk0[:], mk_d[0].rearrange("(t p) s -> p t s", p=128))

            # ---------------- Phase 1: projections ----------------
            with tc.tile_pool(name="xpool", bufs=2) as XP, \
                 tc.tile_pool(name="wpool", bufs=2) as WP, \
                 tc.tile_pool(name="ph1ps", bufs=2, space="PSUM") as PR:
                # Q and K: out[pair-dk 128, sq] += w.T @ x, bias via K=1 matmul
                for xd, wd, brow, dstT in ((xq_d, wq_d, bq_sb, qT),
                                           (xk_d, wk_d, bk_sb, kT)):
                    w_sb = WP.tile([128, KTN, 512], F32R, name="w_sb")
                    for k in range(KTN):
                        nc.sync.dma_start(w_sb[:, k, :], wd[k * 128:(k + 1) * 128, :])
                    for n in range(NJ):
                        x_sb = XP.tile([128, KTN, SQB], F32R, name="x_sb")
                        for k in range(KTN):
                            nc.sync.dma_start(
                                x_sb[:, k, :],
                                xd[k * 128:(k + 1) * 128, n * SQB:(n + 1) * SQB])
                        for hp in range(PAIRS):
                            ps = PR.tile([128, SQB], F32, name="ps_qk")
                            for k in range(KTN):
                                nc.tensor.matmul(
                                    ps[:], w_sb[:, k, hp * 128:(hp + 1) * 128],
                                    x_sb[:, k, :], start=(k == 0), stop=False)
                            nc.tensor.matmul(
                                ps[:], brow[0:1, hp * 128:(hp + 1) * 128],
                                ones[0:1, :], start=False, stop=True)
                            nc.scalar.activation(
                                dstT[:, hp, n * SQB:(n + 1) * SQB], ps[:], AF.Copy)

                # V: out[sk 128, head, dv] += x.T @ wv, bias via K=1 matmul
                wv_sb = WP.tile([128, KTN, 512], F32R, name="w_sb")
                for k in range(KTN):
                    nc.sync.dma_start(wv_sb[:, k, :], wv_d[k * 128:(k + 1) * 128, :])
                for c in range(4):
                    x_sb = XP.tile([128, KTN, SQB], F32R, name="x_sb")
                    for k in range(KTN):
                        nc.sync.dma_start(
                            x_sb[:, k, :],
                            xv_d[k * 128:(k + 1) * 128, c * SQB:(c + 1) * SQB])
                    for stl in range(4):
                        st = c * 4 + stl
                        ps = PR.tile([128, HPC, DV], F32, name="ps_v")
                        for k in range(KTN):
                            nc.tensor.matmul(
                                ps[:], x_sb[:, k, stl * 128:(stl + 1) * 128],
                                wv_sb[:, k, :], start=(k == 0), stop=False)
                        nc.tensor.matmul(ps[:], ones[0:1, 0:128],
                                         bv_sb[0:1, :],
                                         start=False, stop=True)
                        nc.vector.tensor_copy(vA[:, st, :, 0:DV], ps[:])

            # ---------------- Phase 2+3: attention + out-projection ----------------
            # opps first: it inherits phase-1's PSUM banks but is first used a
            # full j-block later, so the first scores/PV matmuls start clean.
            with tc.tile_pool(name="opps", bufs=2, space="PSUM") as OPP, \
                 tc.tile_pool(name="pvps", bufs=2, space="PSUM") as PVP, \
                 tc.tile_pool(name="scps", bufs=2, space="PSUM") as SCP:

                def tail_steps(j, rc8, oU, oN):
                    steps = []
                    for hl in range(HPC):
                        hp, r = divmod(hl, 2)
                        pb = 64 * r

                        def s_bc(hl=hl, hp=hp, pb=pb):
                            bc = OPP.tile([128, SQB], F32, name="op")
                            nc.tensor.matmul(bc[0:DV, :],
                                             sel[:, hl * DV:(hl + 1) * DV],
                                             rc8[:], start=True, stop=True)
                            nc.vector.tensor_mul(oN[pb:pb + DV, hp, :],
                                                 oU[pb:pb + DV, hp, :],
                                                 bc[0:DV, :])
                        steps.append(s_bc)
                    for stl in range(4):
                        for nn in range(2):
                            def s_op(stl=stl, nn=nn):
                                st = 4 * j + stl
                                op = OPP.tile([128, SQB], F32, name="op")
                                for hp in range(PAIRS):
                                    nc.tensor.matmul(
                                        op[:],
                                        oN[:, hp, stl * 128:(stl + 1) * 128],
                                        wo_sb[:, hp, nn * SQB:(nn + 1) * SQB],
                                        start=(hp == 0), stop=(hp == PAIRS - 1))
                                ob = OBP.tile([128, SQB], F32, name="ob")
                                nc.vector.tensor_copy(ob[:], op[:])
                                nc.sync.dma_start(
                                    out_d[st * 128:(st + 1) * 128,
                                          nn * SQB:(nn + 1) * SQB], ob[:])
                            steps.append(s_op)
                    return steps

                def emit_recip(denj):
                    rc8 = RCP.tile([HPC, SQB], F32R, name="rc8")
                    with nc.allow_low_precision(reason="f32r recip"):
                        nc.vector.reciprocal(rc8[:], denj[:])
                    return rc8

                pend = None
                for j in range(NJ):
                    if j == 0:
                        msk = msk0
                    else:
                        msk = MP.tile([128, NT, SQB], BF16, name="msk")
                        nc.sync.dma_start(
                            msk[:], mk_d[j].rearrange("(t p) s -> p t s", p=128))
                    denj = DNP.tile([HPC, SQB], F32R, name="denj")
                    oU = OUP.tile([128, PAIRS, SQB], BF16, name="oU")
                    oN = ONP.tile([128, PAIRS, SQB], BF16, name="oN")
                    steps = []
                    for hl in range(HPC):
                        hp, r = divmod(hl, 2)
                        pb = 64 * r
                        pv = PVP.tile([DV + 1, SQB], F32, name="pv")
                        # software pipeline: scores MMs for tt enqueue on the PE
                        # before PV MMs for tt-1, so the PE never head-of-line
                        # blocks on exp/mask of the previous tile.
                        prev_pt = None
                        for tt in range(NT // 2):
                            sc = SCP.tile([128, 2, SQB], F32, name="sc")
                            for u in range(2):
                                t = 2 * tt + u
                                nc.tensor.matmul(
                                    sc[:, u, :],
                                    kT[pb:pb + DK, hp, t * 128:(t + 1) * 128],
                                    qT[pb:pb + DK, hp, j * SQB:(j + 1) * SQB],
                                    start=True, stop=True)
                            if prev_pt is not None:
                                ppt, ptt = prev_pt
                                for u in range(2):
                                    nc.tensor.matmul(
                                        pv[:], vA[:, 2 * ptt + u, hl, :],
                                        ppt[:, u, :],
                                        start=(ptt == 0 and u == 0), stop=False)
                            pt = PTP.tile([128, 2, SQB], BF16, name="pt")
                            nc.scalar.activation(pt[:], sc[:], AF.Exp, scale=0.125)
                            nc.vector.tensor_mul(pt[:], pt[:],
                                                 msk[:, 2 * tt:2 * tt + 2, :])
                            prev_pt = (pt, tt)
                            # drip-feed the previous block's normalization +
                            # out-projection between score tiles
                            if pend is not None:
                                gi = hl * (NT // 2) + tt
                                if gi == 6:
                                    rc8 = emit_recip(pend[1])
                                    steps = tail_steps(pend[0], rc8,
                                                       pend[2], pend[3])
                                elif gi >= 12 and steps:
                                    steps.pop(0)()
                        ppt, ptt = prev_pt
                        for u in range(2):
                            nc.tensor.matmul(
                                pv[:], vA[:, 2 * ptt + u, hl, :], ppt[:, u, :],
                                start=False, stop=(u == 1))
                        dtmp = DTP.tile([1, SQB], F32R, name="dtmp")
                        with nc.allow_low_precision(reason="den f32r"):
                            nc.vector.tensor_copy(dtmp[:], pv[DV:DV + 1, :])
                        nc.sync.dma_start(denj[hl:hl + 1, :], dtmp[:])
                        nc.vector.tensor_copy(oU[pb:pb + DV, hp, :], pv[0:DV, :])
                    while steps:
                        steps.pop(0)()
                    pend = (j, denj, oU, oN)
                # final block's tail, nothing left to overlap with
                rc8 = emit_recip(pend[1])
                for s in tail_steps(pend[0], rc8, pend[2], pend[3]):
                    s()
    nc.finalize()
    return nc


def get_nc():
    global _NC
    if _NC is None:
        _NC = _build_nc()
    return _NC


def make_in_maps(q_hidden_inputs, k_hidden_inputs, v_hidden_inputs, mask,
                 wq, bq, wk, bk, wv, bv, wo, bo):
    f32 = np.float32
    bf16 = ml_dtypes.bfloat16
    in_maps = []
    per_batch = []
    sel = np.zeros((HPC, HPC * DV), dtype=f32)
    for hl in range(HPC):
        sel[hl, hl * DV:(hl + 1) * DV] = 1.0
    for b in range(B):
        xqT = np.ascontiguousarray(q_hidden_inputs[b].T, dtype=f32)
        xkT = np.ascontiguousarray(k_hidden_inputs[b].T, dtype=f32)
        xvT = np.ascontiguousarray(v_hidden_inputs[b].T, dtype=f32)
        maskT = mask[b].T.astype(bf16)                        # [sk, sq]
        maskJ = np.ascontiguousarray(
            maskT.reshape(S, NJ, SQB).transpose(1, 0, 2))     # [j, sk, 512]
        per_batch.append((xqT, xkT, xvT, maskJ))
    for c in range(2 * B):
        b, g = divmod(c, 2)
        xqT, xkT, xvT, maskJ = per_batch[b]
        hs = slice(g * HPC, (g + 1) * HPC)
        in_maps.append({
            "xqT": xqT, "xkT": xkT, "xvT": xvT, "maskJ": maskJ,
            "wq": np.ascontiguousarray(
                wq[hs].transpose(1, 0, 2).reshape(HID, 512), dtype=f32),
            "wk": np.ascontiguousarray(
                wk[hs].transpose(1, 0, 2).reshape(HID, 512), dtype=f32),
            "wv": np.ascontiguousarray(
                wv[hs].transpose(1, 0, 2).reshape(HID, 512), dtype=f32),
            "bq": np.ascontiguousarray(bq[hs].reshape(1, 512), dtype=f32),
            "bk": np.ascontiguousarray(bk[hs].reshape(1, 512), dtype=f32),
            "bv": np.ascontiguousarray(bv[hs].reshape(1, 512), dtype=f32),
            "wo": np.ascontiguousarray(
                wo[g * 512:(g + 1) * 512, :].reshape(PAIRS, 128, HID)
            ).astype(bf16),
            "sel": sel,
        })
    return in_maps


def assemble(results, bo):
    out = np.empty((B, S, HID), dtype=np.float32)
    for b in range(B):
        out[b] = results[2 * b]["out"] + results[2 * b + 1]["out"] \
            + bo.astype(np.float32)[None, :]
    return out


def run(inputs, trace=False, **kw):
    nc = get_nc()
    in_maps = make_in_maps(**inputs)
    bkr = run_bass_kernel_spmd(nc, in_maps, list(range(2 * B)), trace=trace, **kw)
    return assemble(bkr.results, np.asarray(inputs["bo"])), bkr


def kernel(**inputs):
    out, _ = run(inputs, trace=False)
    return out
